# revision 10
# baseline (speedup 1.0000x reference)
"""Trainium2 Bass kernel for ColBERT negative-CE loss (8 NeuronCores).

Sharding: data-parallel over query batches (16 per core); doc_embeddings
replicated to every core. Each core computes per-query-token maxes over doc
tokens for all 128 doc batches, plus its own negative-doc maxes. The tiny
O(B^2) tail (token sums, softplus, CE) runs on host in float64.

Doc layout is token-interleaved (column = token*8 + batch) so that every
level of the max-fold tree is a FLAT contiguous tensor_tensor max: level k
folds [0:W] against [W:2W], keeping same-batch partners aligned with no
strided sub-rows (DVE row-boundary overheads avoided, bf16 TT runs 2x).

Per chunk (8 doc batches), 4 PSUM quad-tiles [128, 2048] (one per query
group g, 4 matmuls each, stationary weights reused within the quad):
  - g0 quad (most chunks): DVE reduce_max straight from PSUM (f32, 1x).
  - remaining quads: ScalarE copies PSUM -> bf16 SBUF; DVE runs the flat
    TT-max fold chain (per-quad L1, then batched L2..L8 across the chunk).

A PE warmup burst (dummy matmuls during the DMA prefetch window) holds the
tensor engine busy ~4us so the HAM clock gate releases to 2.4 GHz before
the real work starts.

Self-contained: hardcodes shapes from the problem spec.
"""

import numpy as np

_B, _Nq, _Nd, _D = 128, 32, 256, 128
_M = 8          # cores
_BL = _B // _M  # query batches per core = 16
_T = 0.02

_ND_CHUNKS = 12  # chunks 0..11 have a direct-reduce g0 quad; rest all-evac

_CACHE = {}


def _build_program():
    """Trace + compile the per-core Bass program (same program for all cores)."""
    from contextlib import ExitStack

    import concourse.bacc as bacc
    import concourse.tile as tile
    from concourse import mybir

    f32 = mybir.dt.float32
    # NB: float16 crashes at full scale on this runtime; bfloat16 is safe
    b16 = mybir.dt.bfloat16

    nc = bacc.Bacc("TRN2", target_bir_lowering=False, debug=False, num_devices=_M)

    # DRAM I/O (per core)
    qT = nc.dram_tensor("qT", [128, 512], b16, kind="ExternalInput").ap()
    docc = nc.dram_tensor("docc", [16, 128, 2048], b16, kind="ExternalInput").ap()
    negc = nc.dram_tensor("negc", [2, 128, 2048], b16, kind="ExternalInput").ap()
    maxcol = nc.dram_tensor("maxcol", [128, 512], f32, kind="ExternalOutput").ap()
    negmax = nc.dram_tensor("negmax", [128, 16], f32, kind="ExternalOutput").ap()

    with tile.TileContext(nc) as tc, ExitStack() as ctx:
        singles = ctx.enter_context(tc.tile_pool(name="singles", bufs=1))
        megapool = ctx.enter_context(tc.tile_pool(name="mega", bufs=3))
        foldpool = ctx.enter_context(tc.tile_pool(name="fold", bufs=3))
        pspool = ctx.enter_context(tc.tile_pool(name="ps", bufs=2, space="PSUM"))

        qt = singles.tile([128, 512], b16)
        nc.sync.dma_start(out=qt, in_=qT)

        # prefetch the whole doc set + negs into SBUF (fits comfortably);
        # first two chunks go out ahead of the neg loads
        dts = [
            singles.tile([128, 2048], b16, tag=f"dt{ch}", name=f"dt{ch}")
            for ch in range(16)
        ]
        ng = [
            singles.tile([128, 2048], b16, tag=f"ng{n}", name=f"ng{n}")
            for n in range(2)
        ]
        nc.sync.dma_start(out=dts[0], in_=docc[0])
        nc.sync.dma_start(out=dts[1], in_=docc[1])
        nc.sync.dma_start(out=ng[0], in_=negc[0])
        nc.sync.dma_start(out=ng[1], in_=negc[1])
        for ch in range(2, 16):
            nc.sync.dma_start(out=dts[ch], in_=docc[ch])

        mc = singles.tile([128, 512], f32, tag="mc")
        nm = singles.tile([128, 16], f32, tag="nm")

        # PE warmup: ~16 back-to-back dummy matmuls (~4us cold) while the doc
        # DMAs land, so the HAM clock gate sees a full busy window and lifts
        # the PE to 2.4 GHz before chunk 0 is processed.
        wz = singles.tile([128, 512], b16, tag="wz")
        nc.vector.memset(wz, 0)
        wps = pspool.tile([128, 2048], f32, tag="ps")
        for w in range(16):
            nc.tensor.matmul(
                wps[:, (w % 4) * 512 : (w % 4 + 1) * 512],
                wz[:, 0:128],
                wz[:, 0:512],
                start=True,
                stop=True,
            )

        def do_chunk(ch):
            has_d = ch < _ND_CHUNKS
            ns = 3 if has_d else 4  # quads going through scalar evac
            # per-quad L1 fold outputs (flat, staggered right after each copy)
            f1 = foldpool.tile([128, 4096], b16, tag="f1")
            for g in range(4):
                ps = pspool.tile([128, 2048], f32, tag="ps")
                for j in range(4):
                    nc.tensor.matmul(
                        ps[:, j * 512 : (j + 1) * 512],
                        qt[:, g * 128 : (g + 1) * 128],
                        dts[ch][:, j * 512 : (j + 1) * 512],
                        start=True,
                        stop=True,
                    )
                if has_d and g == 0:
                    # interleaved layout: batch = col % 8 -> strided view
                    nc.vector.reduce_max(
                        out=mc[:, ch * 32 : ch * 32 + 8],
                        in_=ps[:].rearrange("p (t c) -> p c t", c=8),
                        axis=mybir.AxisListType.X,
                    )
                else:
                    k = g - 1 if has_d else g
                    mega = megapool.tile([128, 2048], b16, tag=f"mega{k}")
                    nc.scalar.copy(out=mega, in_=ps)
                    # L1: flat fold 2048 -> 1024 (same-batch partners by layout)
                    nc.vector.tensor_max(
                        out=f1[:, k * 1024 : (k + 1) * 1024],
                        in0=mega[:, 0:1024],
                        in1=mega[:, 1024:2048],
                    )
            base = ch * 32 + (8 if has_d else 0)
            # batched flat chain over the chunk's ns quads: 1024 -> 8 each
            v = f1[:, 0 : ns * 1024].rearrange("p (q w) -> p q w", w=1024)
            f2 = foldpool.tile([128, 4, 512], b16, tag="f2")
            nc.vector.tensor_max(
                out=f2[:, 0:ns, :], in0=v[:, :, 0:512], in1=v[:, :, 512:1024]
            )
            f3 = foldpool.tile([128, 4, 256], b16, tag="f3")
            nc.vector.tensor_max(
                out=f3[:, 0:ns, :], in0=f2[:, 0:ns, 0:256], in1=f2[:, 0:ns, 256:512]
            )
            f4 = foldpool.tile([128, 4, 128], b16, tag="f4")
            nc.vector.tensor_max(
                out=f4[:, 0:ns, :], in0=f3[:, 0:ns, 0:128], in1=f3[:, 0:ns, 128:256]
            )
            f5 = foldpool.tile([128, 4, 64], b16, tag="f5")
            nc.vector.tensor_max(
                out=f5[:, 0:ns, :], in0=f4[:, 0:ns, 0:64], in1=f4[:, 0:ns, 64:128]
            )
            f6 = foldpool.tile([128, 4, 32], b16, tag="f6")
            nc.vector.tensor_max(
                out=f6[:, 0:ns, :], in0=f5[:, 0:ns, 0:32], in1=f5[:, 0:ns, 32:64]
            )
            f7 = foldpool.tile([128, 4, 16], b16, tag="f7")
            nc.vector.tensor_max(
                out=f7[:, 0:ns, :], in0=f6[:, 0:ns, 0:16], in1=f6[:, 0:ns, 16:32]
            )
            # final level writes f32 straight into mc (tiny, 1x is fine)
            nc.vector.tensor_max(
                out=mc[:, base : base + ns * 8].rearrange("p (q w) -> p q w", w=8),
                in0=f7[:, 0:ns, 0:8],
                in1=f7[:, 0:ns, 8:16],
            )

        def do_neg(n):
            # one quad covers ng[n] = local neg batches 8n..8n+7; stationary
            # switches q-group mid-quad (g=2n for first half, 2n+1 for second)
            ps = pspool.tile([128, 2048], f32, tag="ps")
            for h in range(4):
                g = 2 * n + h // 2
                nc.tensor.matmul(
                    ps[:, h * 512 : (h + 1) * 512],
                    qt[:, g * 128 : (g + 1) * 128],
                    ng[n][:, h * 512 : (h + 1) * 512],
                    start=True,
                    stop=True,
                )
            # negc keeps the batch-major layout (each matmul's stationary g
            # must match the batches in its column slice), so blocks are
            # contiguous 256-token runs here
            nc.vector.reduce_max(
                out=nm[:, n * 8 : (n + 1) * 8],
                in_=ps[:].rearrange("p (c s) -> p c s", s=256),
                axis=mybir.AxisListType.X,
            )

        for ch in range(8):
            do_chunk(ch)
        do_neg(0)
        do_chunk(8)
        # first half of mc is complete after chunk 7; stream it out on the
        # (otherwise idle) SWDGE queue
        nc.gpsimd.dma_start(out=maxcol[:, 0:256], in_=mc[:, 0:256])
        do_neg(1)
        nc.gpsimd.dma_start(out=negmax, in_=nm)
        for ch in range(9, 16):
            do_chunk(ch)
        nc.gpsimd.dma_start(out=maxcol[:, 256:512], in_=mc[:, 256:512])

    nc.compile()
    return nc


def _get_program():
    if "nc" not in _CACHE:
        _CACHE["nc"] = _build_program()
    return _CACHE["nc"]


def _colmap():
    """mc column (ch*32 + g*8 + blk) -> scores column (g*128 + ch*8 + blk)."""
    cmap = np.empty(512, dtype=np.int64)
    for ch in range(16):
        for g in range(4):
            for b in range(8):
                cmap[ch * 32 + g * 8 + b] = g * 128 + ch * 8 + b
    return cmap


def prep_inputs(query_embeddings, doc_embeddings, neg_doc_embeddings):
    """Host-side sharding + layout prep -> per-core input maps."""
    import ml_dtypes

    bf = ml_dtypes.bfloat16
    q = np.asarray(query_embeddings, dtype=np.float32).astype(bf)
    d = np.asarray(doc_embeddings, dtype=np.float32).astype(bf)
    n = np.asarray(neg_doc_embeddings, dtype=np.float32).astype(bf)

    # docs: [B, Nd, D] -> [16, D, token, batch] (token-interleaved columns)
    docc = np.ascontiguousarray(
        d.reshape(16, 8, _Nd, _D).transpose(0, 3, 2, 1)
    ).reshape(16, 128, 2048)

    in_maps = []
    for i in range(_M):
        qs = q[i * _BL : (i + 1) * _BL]  # [16, 32, 128]
        qT = np.ascontiguousarray(qs.transpose(2, 0, 1)).reshape(128, 512)
        ns = n[i * _BL : (i + 1) * _BL]  # [16, 256, 128]
        negc = np.ascontiguousarray(
            ns.reshape(2, 8, _Nd, _D).transpose(0, 3, 1, 2)
        ).reshape(2, 128, 2048)
        in_maps.append({"qT": qT, "docc": docc, "negc": negc})
    return in_maps


def postprocess(results):
    """Combine per-core outputs into the scalar loss (float64 host math)."""
    cmap = _colmap()
    scores = np.empty((_B, _B), dtype=np.float64)
    neg = np.empty((_B,), dtype=np.float64)
    for i in range(_M):
        mc_raw = np.asarray(results[i]["maxcol"], dtype=np.float64)  # [128, 512]
        nm = np.asarray(results[i]["negmax"], dtype=np.float64)  # [128, 16]
        mc = np.empty_like(mc_raw)
        mc[:, cmap] = mc_raw
        # mc[p, g*128+c], p = m*32+n, local batch = 4g+m
        s4 = mc.reshape(4, 32, 4, 128).sum(axis=1)  # [m, g, c]
        scores[i * _BL : (i + 1) * _BL] = s4.transpose(1, 0, 2).reshape(16, 128)
        # nm[32*(b%4)+t, n*8+b] = token-t max for local neg batch 8n+b
        for n_ in range(2):
            for b in range(8):
                neg[i * _BL + 8 * n_ + b] = nm[
                    32 * (b % 4) : 32 * (b % 4) + 32, n_ * 8 + b
                ].sum()

    t = _T
    pos = np.diag(scores)
    term1 = np.logaddexp(0.0, (neg - pos) / t).mean()
    lg = scores / t
    m_ = lg.max(axis=1)
    lse = m_ + np.log(np.exp(lg - m_[:, None]).sum(axis=1))
    ce = (lse - np.diag(lg)).mean()
    return np.float32((term1 + ce) / 2.0)


def run_device(in_maps, **kwargs):
    from concourse.bass_utils import run_bass_kernel_spmd

    nc = _get_program()
    return run_bass_kernel_spmd(nc, in_maps, list(range(_M)), **kwargs)


def kernel(query_embeddings, doc_embeddings, neg_doc_embeddings):
    in_maps = prep_inputs(query_embeddings, doc_embeddings, neg_doc_embeddings)
    res = run_device(in_maps)
    return postprocess(res.results)
